# revision 26
# baseline (speedup 1.0000x reference)
"""Batched matrix-attention scores kernel for Trainium2 (8 NeuronCores).

Computes scores[b, i, j] = sum_d m1[b, i, d] * m2[b, j, d]
  (i.e. jnp.einsum('bid,bjd->bij', matrix_1, matrix_2))
with B=16, R1=R2=2048, D=256, fp32 in/out.

Sharding: data-parallel over batch - 2 batches per core on 8 cores.

Per-core HBM traffic is 8 MiB of loads + 32 MiB of stores; a single
HWDGE queue sustains ~420 GB/s, so the roofline is ~100 us. The
schedule is built to keep the DMA queues fed continuously:

  - b0 loads are split across both HWDGE rings (sync+scalar) in
    512 KiB quad-chunks; b1 loads follow on the scalar ring while
    stores own the sync ring.
  - Operands are PE-transposed (matmul transpose mode with identity)
    into D-on-partitions layout mT[d, dc, row]; transposes are packed
    4-to-a-PSUM-bank so one [128,512] copy evacuates a whole quad, and
    quads chase the load chunks so the first matmul block starts ~8us.
  - Each 128-row output tile takes 8 matmuls (2 d-chunks x 4 j-chunks
    of N=512, two 2-bank PSUM tiles); PSUM is evacuated split across
    DVE and ACT (one [128,1024] copy each) so store production stays
    above the DMA drain rate; output stored in 1 MiB blocks on the
    sync ring.
  - Remaining transpose quads (m1 of the current batch, both matrices
    of the next) are trickled between matmul blocks so the PE never
    lets the store queue drain.

Operands use dt.float32r (fp32 bits, full-rate single-pass PE matmul;
~2^-11 input mantissa truncation). Accumulation stays fp32 in PSUM.
"""

from contextlib import ExitStack

import numpy as np

import concourse.bass as bass
import concourse.mybir as mybir
import concourse.tile as tile
from concourse import bacc
from concourse.bass_utils import run_bass_kernel_spmd

F32 = mybir.dt.float32
F32R = mybir.dt.float32r

NCORES = 8
B, R1, R2, D = 16, 2048, 2048, 256
BPC = B // NCORES  # batches per core
P = 128
NJ_TILE = 512  # matmul free dim (one fp32 PSUM bank)
NJ = R2 // NJ_TILE  # j-chunks per row-block
NT = R1 // P  # 128-row tiles per batch
DC = D // P  # contraction chunks
NQ = NT // 4  # transpose quads (4 row-blocks) per (matrix, dc)
WARMUP_T = 10  # HAM warmup transposes


def _build_tile_kernel(ctx: ExitStack, tc: tile.TileContext, m1, m2, out):
    nc = tc.nc

    const_pool = ctx.enter_context(tc.tile_pool(name="const", bufs=1))
    # identity built on-chip right after the startup barrier (~3.5us) —
    # no DMA dependency, so HAM warmup transposes can start immediately
    ident = const_pool.tile([P, P], F32R)
    ident_f = const_pool.tile([P, P], F32, tag="identf", name="ident_f")
    nc.gpsimd.memset(ident_f, 1.0)
    nc.gpsimd.affine_select(
        out=ident_f,
        in_=ident_f,
        compare_op=mybir.AluOpType.is_equal,
        fill=0.0,
        base=0,
        # ident_f[x, y] = (x - y == 0) ? 1.0 : 0.0
        pattern=[[-1, P]],
        channel_multiplier=1,
    )
    nc.vector.tensor_copy(ident, ident_f)  # f32 -> f32r rounding copy

    nat_pool = ctx.enter_context(tc.tile_pool(name="nat", bufs=1))
    mt_pool = ctx.enter_context(tc.tile_pool(name="mt", bufs=1))
    tpsum = ctx.enter_context(tc.tile_pool(name="tpsum", bufs=2, space="PSUM"))
    mpsum = ctx.enter_context(tc.tile_pool(name="mpsum", bufs=3, space="PSUM"))
    outp = ctx.enter_context(tc.tile_pool(name="outp", bufs=8))

    # nat buffers are SHARED across batches in CROSS tags (A: m2b0+m1b1,
    # B: m1b0+m2b1, bufs=1): a b1 chunk-q load then carries a WAR
    # dependency on b0's quad-q transposes of the tag partner — a
    # scheduler-proof gate that releases b1 load traffic at ~13us (once
    # the ramp-critical b0 data has been consumed) but well before the
    # evac-heavy matmul phase where the SWDGE queue gets starved.
    #
    # m1 is loaded CONTIGUOUSLY (partition p holds 4 consecutive DRAM
    # rows 4p+t of its 512-row chunk -> 4 KiB descriptor lines instead
    # of 1 KiB); the resulting row permutation is absorbed by the store
    # AP (partition stride 4 rows), which keeps 8 KiB store lines. m2
    # feeds the matmul j-columns, so it must stay in natural row order
    # (1 KiB lines).
    # per-chunk nat tiles, all shaped [P, 4, D]; chunk q of (m2,b0) shares
    # a buffer with chunk q of (m1,b1) and vice versa, so each b1 chunk
    # load WAR-waits on exactly its partner chunk's 8 transposes
    nat = {}
    mt = {}
    for b in range(BPC):
        for name in ("m2", "m1"):
            grp = "a" if (name == "m2") == (b == 0) else "b"
            for q in range(NQ):
                nat[(name, b, q)] = nat_pool.tile(
                    [P, 4, D], F32R, tag=f"nat_{grp}_{q}", name=f"nat_{name}_{b}_{q}"
                )
            mt[(name, b)] = mt_pool.tile(
                [P, DC, R1], F32R, tag=f"mt_{name}_{b}", name=f"mt_{name}_{b}"
            )

    def load_chunk(eng, name, b, q):
        """One 512 KiB quad-chunk (4 row-blocks) of a matrix into nat."""
        if name == "m2":
            eng.dma_start(
                nat[(name, b, q)],
                m2[b].rearrange("(o p) d -> p o d", p=P)[:, q * 4 : (q + 1) * 4, :],
            )
        else:
            eng.dma_start(
                nat[(name, b, q)],
                m1[b].rearrange("(q p t) d -> p q t d", p=P, t=4)[:, q],
            )

    t_toggle = [0]

    def t_quad(name, b, q, dc):
        """Transpose 4 row-blocks (one d-chunk) into one PSUM bank, then
        evacuate with a single [128,512] copy on alternating engines."""
        tp = tpsum.tile([P, NJ_TILE], F32R, tag="tp", name=f"tp_{name}_{b}_{q}_{dc}")
        for k in range(4):
            src = nat[(name, b, q)][:, k, dc * P : (dc + 1) * P]
            nc.tensor.transpose(tp[:, k * P : (k + 1) * P], src, ident)
        dst = mt[(name, b)][:, dc, q * NJ_TILE : (q + 1) * NJ_TILE]
        if t_toggle[0] % 2 == 0:
            nc.vector.tensor_copy(dst, tp)
        else:
            nc.scalar.copy(dst, tp)
        t_toggle[0] += 1

    def t_pair(name, b, q):
        for dc in range(DC):
            t_quad(name, b, q, dc)

    stages = {}

    def mm_half(b, it, half, split_store=False):
        """Half of a 128-row output tile (j-halves of 1024): 4 matmuls into
        one 2-bank PSUM tile, evacuated on DVE (half 0) / ACT (half 1).
        split_store: store this half on its own (512 KiB); else the full
        1 MiB row-block is stored on sync once both halves are staged."""
        m2T, m1T = mt[("m2", b)], mt[("m1", b)]
        if (b, it) not in stages:
            stages[(b, it)] = outp.tile(
                [P, R2], F32, tag="stage", name=f"stage_{b}_{it}"
            )
        stage = stages[(b, it)]
        ps = mpsum.tile([P, 2 * NJ_TILE], F32, tag="mm", name=f"mps_{b}_{it}_{half}")
        for jl in range(2):
            jc = half * 2 + jl
            for dc in range(DC):
                nc.tensor.matmul(
                    ps[:, jl * NJ_TILE : (jl + 1) * NJ_TILE],
                    m1T[:, dc, it * P : (it + 1) * P],
                    m2T[:, dc, jc * NJ_TILE : (jc + 1) * NJ_TILE],
                    start=(dc == 0),
                    stop=(dc == DC - 1),
                )
        lo, hi = half * 2 * NJ_TILE, (half + 1) * 2 * NJ_TILE
        dst = stage[:, lo:hi]
        if half == 0:
            nc.vector.tensor_copy(dst, ps)
        else:
            nc.scalar.copy(dst, ps)
        # m1's contiguous load layout maps psum partition p to DRAM row
        # qq*512 + 4p + t -- the store AP un-permutes (8 KiB lines kept)
        qq, t = divmod(it, 4)
        orows = out[b].rearrange("(q p t) j -> p q t j", p=P, t=4)[:, qq, t, :]
        if split_store:
            eng = nc.scalar if (half == 1 and it == NT - 1 and b == BPC - 1) else nc.sync
            eng.dma_start(orows[:, lo:hi], stage[:, lo:hi])
        elif half == 1:
            nc.sync.dma_start(orows, stage)

    def mm_block(b, it, split_store=False):
        mm_half(b, it, 0, split_store)
        mm_half(b, it, 1, split_store)

    # ---- loads ----
    # b0 is ramp-critical: split across sync+scalar rings with the
    # first-store-critical chunks (m2 q0/q1, m1 q0) in front. All b1
    # loads ride the otherwise-idle gpsimd (SWDGE) queue; the shared nat
    # buffers gate them (WAR) behind b0's transposes.
    # the three chunks gating the first store (m2 q0/q1, m1 q0) each ride
    # position 1 of a different queue; m1b0 q0 takes the gpsimd queue,
    # which is otherwise idle until the gated b1 loads release
    for name, b, q in (("m2", 0, 0), ("m2", 0, 2), ("m1", 0, 1), ("m1", 0, 3)):
        load_chunk(nc.sync, name, b, q)
    for name, b, q in (("m2", 0, 1), ("m2", 0, 3), ("m1", 0, 2)):
        load_chunk(nc.scalar, name, b, q)
    load_chunk(nc.gpsimd, "m1", 0, 0)
    # m1b1 next on the SWDGE queue: released earliest (gated by m2b0's
    # transposes) and must finish before the evac-heavy phase starves Q0
    for q in range(NQ):
        load_chunk(nc.gpsimd, "m1", 1, q)
    for q in range(NQ):
        load_chunk(nc.gpsimd, "m2", 1, q)

    # ---- HAM warmup: dummy transposes on the identity ----
    for w in range(WARMUP_T):
        wtp = tpsum.tile([P, NJ_TILE], F32R, tag="tp", name=f"warm_{w}")
        nc.tensor.transpose(wtp[:, 0:P], ident, ident)

    # ---- ramp: transposes chase the load chunks; blocks 0-1 run and
    # store in j-halves so the first store needs only half of m2T.
    # All m1b0 pairs go early: each one releases a cross-tag-gated m2b1
    # load chunk, and those must be on the wire before the evac-heavy
    # phase starves the SWDGE queue. ----
    t_pair("m2", 0, 0)
    t_pair("m2", 0, 1)
    t_pair("m1", 0, 0)
    mm_half(0, 0, 0, split_store=True)
    mm_half(0, 1, 0, split_store=True)
    t_pair("m1", 0, 1)
    t_pair("m2", 0, 2)
    t_pair("m2", 0, 3)
    mm_half(0, 0, 1, split_store=True)
    mm_half(0, 1, 1, split_store=True)
    t_pair("m1", 0, 2)
    t_pair("m1", 0, 3)

    # b1 m2 quads (8 singles) after b0 blocks 7..14
    b1_m2 = [("m2", 1, q, dc) for q in range(NQ) for dc in range(DC)]
    after_b0_late = {
        15: [("m1", 1, 0), ("m1", 1, 1)],
    }
    after_b1_blocks = {
        1: [("m1", 1, 2)],
        3: [("m1", 1, 3)],
    }

    for it in range(2, NT):
        mm_block(0, it)
        if 7 <= it <= 14:
            name, b, q, dc = b1_m2[it - 7]
            t_quad(name, b, q, dc)
        for name, b, q in after_b0_late.get(it, []):
            t_pair(name, b, q)

    for it in range(NT):
        # last row-block stores in halves on both rings to shorten the
        # final drain
        mm_block(1, it, split_store=(it == NT - 1))
        for name, b, q in after_b1_blocks.get(it, []):
            t_pair(name, b, q)


_NC_CACHE = None


def _build():
    global _NC_CACHE
    if _NC_CACHE is not None:
        return _NC_CACHE
    nc = bacc.Bacc(
        "TRN2", target_bir_lowering=False, debug=False, num_devices=NCORES
    )
    m1 = nc.dram_tensor("m1", [BPC, R1, D], F32R, kind="ExternalInput").ap()
    m2 = nc.dram_tensor("m2", [BPC, R2, D], F32R, kind="ExternalInput").ap()
    out = nc.dram_tensor("out", [BPC, R1, R2], F32, kind="ExternalOutput").ap()
    with tile.TileContext(nc) as tc:
        with ExitStack() as ctx:
            _build_tile_kernel(ctx, tc, m1, m2, out)
    nc.compile()
    _NC_CACHE = nc
    return nc


def kernel(matrix_1: np.ndarray, matrix_2: np.ndarray, **run_kwargs) -> np.ndarray:
    m1 = np.ascontiguousarray(np.asarray(matrix_1, dtype=np.float32))
    m2 = np.ascontiguousarray(np.asarray(matrix_2, dtype=np.float32))
    assert m1.shape == (B, R1, D) and m2.shape == (B, R2, D)

    nc = _build()
    in_maps = [
        {
            "m1": m1[i * BPC : (i + 1) * BPC],
            "m2": m2[i * BPC : (i + 1) * BPC],
        }
        for i in range(NCORES)
    ]
    res = run_bass_kernel_spmd(
        nc, in_maps, core_ids=list(range(NCORES)), **run_kwargs
    )
    out = np.empty((B, R1, R2), dtype=np.float32)
    for i in range(NCORES):
        out[i * BPC : (i + 1) * BPC] = res.results[i]["out"]
    if run_kwargs:
        kernel.last_result = res
    return out


# revision 36
# speedup vs baseline: 1.0516x; 1.0516x over previous
"""Batched matrix-attention scores kernel for Trainium2 (8 NeuronCores).

Computes scores[b, i, j] = sum_d m1[b, i, d] * m2[b, j, d]
  (i.e. jnp.einsum('bid,bjd->bij', matrix_1, matrix_2))
with B=16, R1=R2=2048, D=256, fp32 in/out.

Sharding: data-parallel over batch - 2 batches per core on 8 cores.

Per-core HBM traffic is 8 MiB of loads + 32 MiB of stores; a single
HWDGE queue sustains ~420 GB/s, so the roofline is ~100 us. The
schedule is built to keep the DMA queues fed continuously:

  - b0 loads are split across both HWDGE rings (sync+scalar) in
    512 KiB quad-chunks; b1 loads follow on the scalar ring while
    stores own the sync ring.
  - Operands are PE-transposed (matmul transpose mode with identity)
    into D-on-partitions layout mT[d, dc, row]; transposes are packed
    4-to-a-PSUM-bank so one [128,512] copy evacuates a whole quad, and
    quads chase the load chunks so the first matmul block starts ~8us.
  - Each 128-row output tile takes 8 matmuls (2 d-chunks x 4 j-chunks
    of N=512, two 2-bank PSUM tiles); PSUM is evacuated split across
    DVE and ACT (one [128,1024] copy each) so store production stays
    above the DMA drain rate; output stored in 1 MiB blocks on the
    sync ring.
  - Remaining transpose quads (m1 of the current batch, both matrices
    of the next) are trickled between matmul blocks so the PE never
    lets the store queue drain.

Operands use dt.float32r (fp32 bits, full-rate single-pass PE matmul;
~2^-11 input mantissa truncation). Accumulation stays fp32 in PSUM.
"""

from contextlib import ExitStack

import numpy as np

import concourse.bass as bass
import concourse.mybir as mybir
import concourse.tile as tile
from concourse import bacc
from concourse.bass_utils import run_bass_kernel_spmd

F32 = mybir.dt.float32
F32R = mybir.dt.float32r

NCORES = 8
B, R1, R2, D = 16, 2048, 2048, 256
BPC = B // NCORES  # batches per core
P = 128
NJ_TILE = 512  # matmul free dim (one fp32 PSUM bank)
NJ = R2 // NJ_TILE  # j-chunks per row-block
NT = R1 // P  # 128-row tiles per batch
DC = D // P  # contraction chunks
NQ = NT // 4  # transpose quads (4 row-blocks) per (matrix, dc)
WARMUP_T = 10  # HAM warmup transposes


def _build_tile_kernel(ctx: ExitStack, tc: tile.TileContext, m1, m2, out):
    nc = tc.nc

    const_pool = ctx.enter_context(tc.tile_pool(name="const", bufs=1))
    # identity built on-chip right after the startup barrier (~3.5us) —
    # no DMA dependency, so HAM warmup transposes can start immediately
    ident = const_pool.tile([P, P], F32R)
    ident_f = const_pool.tile([P, P], F32, tag="identf", name="ident_f")
    nc.gpsimd.memset(ident_f, 1.0)
    nc.gpsimd.affine_select(
        out=ident_f,
        in_=ident_f,
        compare_op=mybir.AluOpType.is_equal,
        fill=0.0,
        base=0,
        # ident_f[x, y] = (x - y == 0) ? 1.0 : 0.0
        pattern=[[-1, P]],
        channel_multiplier=1,
    )
    nc.vector.tensor_copy(ident, ident_f)  # f32 -> f32r rounding copy

    nat_pool = ctx.enter_context(tc.tile_pool(name="nat", bufs=1))
    mt_pool = ctx.enter_context(tc.tile_pool(name="mt", bufs=1))
    tpsum = ctx.enter_context(tc.tile_pool(name="tpsum", bufs=2, space="PSUM"))
    mpsum = ctx.enter_context(tc.tile_pool(name="mpsum", bufs=3, space="PSUM"))
    outp = ctx.enter_context(tc.tile_pool(name="outp", bufs=8))

    # nat buffers are SHARED across batches in CROSS tags (A: m2b0+m1b1,
    # B: m1b0+m2b1, bufs=1): a b1 chunk-q load then carries a WAR
    # dependency on b0's quad-q transposes of the tag partner — a
    # scheduler-proof gate that releases b1 load traffic at ~13us (once
    # the ramp-critical b0 data has been consumed) but well before the
    # evac-heavy matmul phase where the SWDGE queue gets starved.
    #
    # m1 is loaded CONTIGUOUSLY (partition p holds 4 consecutive DRAM
    # rows 4p+t of its 512-row chunk -> 4 KiB descriptor lines instead
    # of 1 KiB); the resulting row permutation is absorbed by the store
    # AP (partition stride 4 rows), which keeps 8 KiB store lines. m2
    # feeds the matmul j-columns, so it must stay in natural row order
    # (1 KiB lines).
    # per-chunk nat tiles, all shaped [P, 4, D]; chunk q of (m2,b0) shares
    # a buffer with chunk q of (m1,b1) and vice versa, so each b1 chunk
    # load WAR-waits on exactly its partner chunk's 8 transposes
    nat = {}
    mt = {}
    for b in range(BPC):
        for name in ("m2", "m1"):
            grp = "a" if (name == "m2") == (b == 0) else "b"
            for q in range(NQ):
                nat[(name, b, q)] = nat_pool.tile(
                    [P, 4, D], F32R, tag=f"nat_{grp}_{q}", name=f"nat_{name}_{b}_{q}"
                )
            mt[(name, b)] = mt_pool.tile(
                [P, DC, R1], F32R, tag=f"mt_{name}_{b}", name=f"mt_{name}_{b}"
            )

    def load_chunk(eng, name, b, q, half=None):
        """One 512 KiB quad-chunk (4 row-blocks) of a matrix into nat.
        half=0/1 loads only the first/second pair of m1 t-slots (256 KiB)."""
        if name == "m2":
            eng.dma_start(
                nat[(name, b, q)],
                m2[b].rearrange("(o p) d -> p o d", p=P)[:, q * 4 : (q + 1) * 4, :],
            )
        elif half is None:
            eng.dma_start(
                nat[(name, b, q)],
                m1[b].rearrange("(q p t) d -> p q t d", p=P, t=4)[:, q],
            )
        else:
            sl = slice(half * 2, half * 2 + 2)
            eng.dma_start(
                nat[(name, b, q)][:, sl],
                m1[b].rearrange("(q p t) d -> p q t d", p=P, t=4)[:, q, sl],
            )

    t_toggle = [0]

    def t_quad(name, b, q, dc):
        """Transpose 4 row-blocks (one d-chunk) into one PSUM bank, then
        evacuate with a single [128,512] copy on alternating engines."""
        tp = tpsum.tile([P, NJ_TILE], F32R, tag="tp", name=f"tp_{name}_{b}_{q}_{dc}")
        for k in range(4):
            src = nat[(name, b, q)][:, k, dc * P : (dc + 1) * P]
            nc.tensor.transpose(tp[:, k * P : (k + 1) * P], src, ident)
        dst = mt[(name, b)][:, dc, q * NJ_TILE : (q + 1) * NJ_TILE]
        if t_toggle[0] % 2 == 0:
            nc.vector.tensor_copy(dst, tp)
        else:
            nc.scalar.copy(dst, tp)
        t_toggle[0] += 1

    def t_pair(name, b, q):
        for dc in range(DC):
            t_quad(name, b, q, dc)

    def t_duo(name, b, q, slots):
        """Transpose 2 row-slots for BOTH d-chunks into one PSUM bank
        (two [128,256] evac copies) — lets blk0/blk1 start after only
        half of m1's first chunk has landed."""
        tp = tpsum.tile([P, NJ_TILE], F32R, tag="tp", name=f"tpd_{name}_{b}_{q}_{slots[0]}")
        for di in range(DC):
            for si, k in enumerate(slots):
                src = nat[(name, b, q)][:, k, di * P : (di + 1) * P]
                nc.tensor.transpose(
                    tp[:, di * 2 * P + si * P : di * 2 * P + (si + 1) * P], src, ident
                )
        for di in range(DC):
            dst = mt[(name, b)][
                :, di, q * NJ_TILE + slots[0] * P : q * NJ_TILE + (slots[0] + 2) * P
            ]
            if t_toggle[0] % 2 == 0:
                nc.vector.tensor_copy(dst, tp[:, di * 2 * P : (di + 1) * 2 * P])
            else:
                nc.scalar.copy(dst, tp[:, di * 2 * P : (di + 1) * 2 * P])
            t_toggle[0] += 1

    stages = {}

    def mm_half(b, it, half, split_store=False):
        """Half of a 128-row output tile (j-halves of 1024): 4 matmuls into
        one 2-bank PSUM tile, evacuated on DVE (half 0) / ACT (half 1).
        split_store: store this half on its own (512 KiB); else the full
        1 MiB row-block is stored on sync once both halves are staged."""
        m2T, m1T = mt[("m2", b)], mt[("m1", b)]
        if (b, it) not in stages:
            stages[(b, it)] = outp.tile(
                [P, R2], F32, tag="stage", name=f"stage_{b}_{it}"
            )
        stage = stages[(b, it)]
        ps = mpsum.tile([P, 2 * NJ_TILE], F32, tag="mm", name=f"mps_{b}_{it}_{half}")
        for jl in range(2):
            jc = half * 2 + jl
            for dc in range(DC):
                nc.tensor.matmul(
                    ps[:, jl * NJ_TILE : (jl + 1) * NJ_TILE],
                    m1T[:, dc, it * P : (it + 1) * P],
                    m2T[:, dc, jc * NJ_TILE : (jc + 1) * NJ_TILE],
                    start=(dc == 0),
                    stop=(dc == DC - 1),
                )
        lo, hi = half * 2 * NJ_TILE, (half + 1) * 2 * NJ_TILE
        dst = stage[:, lo:hi]
        if half == 0:
            nc.vector.tensor_copy(dst, ps)
        else:
            nc.scalar.copy(dst, ps)
        # m1's contiguous load layout maps psum partition p to DRAM row
        # qq*512 + 4p + t -- the store AP un-permutes (8 KiB lines kept)
        qq, t = divmod(it, 4)
        orows = out[b].rearrange("(q p t) j -> p q t j", p=P, t=4)[:, qq, t, :]
        if split_store:
            eng = nc.scalar if (half == 1 and it == NT - 1 and b == BPC - 1) else nc.sync
            eng.dma_start(orows[:, lo:hi], stage[:, lo:hi])
        elif half == 1:
            nc.sync.dma_start(orows, stage)

    def mm_block(b, it, split_store=False):
        mm_half(b, it, 0, split_store)
        mm_half(b, it, 1, split_store)

    # ---- loads ----
    # The sync queue gets ONLY the three chunks gating the first stores
    # (m2 q0, m1 q0 first half, m2 q2) so it drains by the time the
    # first store's data is staged; the rest of b0 rides scalar, with
    # one chunk (m1 q3, not needed until block 12) on gpsimd. All b1
    # loads follow on the gpsimd (SWDGE) queue; the shared nat buffers
    # gate them (WAR) behind b0's transposes, so they start ~14us and
    # finish before the evac-heavy phase starves SWDGE.
    load_chunk(nc.sync, "m2", 0, 0)
    load_chunk(nc.sync, "m1", 0, 0, half=0)
    load_chunk(nc.sync, "m2", 0, 2)
    load_chunk(nc.scalar, "m2", 0, 1)
    load_chunk(nc.scalar, "m1", 0, 0, half=1)
    load_chunk(nc.scalar, "m2", 0, 3)
    load_chunk(nc.scalar, "m1", 0, 1)
    load_chunk(nc.scalar, "m1", 0, 2)
    load_chunk(nc.gpsimd, "m1", 0, 3)
    # interleave b1's chunks by earliest need: m2b1 feeds transpose quads
    # from b0 block 7 on; m1b1 isn't needed until after b0 block 15
    for name, q in (
        ("m2", 0),
        ("m2", 1),
        ("m1", 0),
        ("m2", 2),
        ("m1", 1),
        ("m2", 3),
        ("m1", 2),
        ("m1", 3),
    ):
        load_chunk(nc.gpsimd, name, 1, q)

    # ---- HAM warmup: dummy transposes on the identity. Fillers are also
    # interleaved into the ramp (see below) so the PE activity window
    # stays busy across load-wait gaps and the first matmuls run at
    # K=8/8 instead of re-throttled 1.2 GHz. ----
    warm_n = [0]

    def fillers(n):
        for _ in range(n):
            wtp = tpsum.tile([P, NJ_TILE], F32R, tag="tp", name=f"warm_{warm_n[0]}")
            nc.tensor.transpose(wtp[:, 0:P], ident, ident)
            warm_n[0] += 1

    fillers(WARMUP_T)

    # ---- ramp: transposes chase the load chunks; blocks 0-1 run and
    # store in j-halves so the first store needs only half of m2T and
    # only m1's first two t-slots (one 256 KiB half-chunk). ----
    t_pair("m2", 0, 0)
    t_pair("m2", 0, 1)
    t_duo("m1", 0, 0, (0, 1))
    mm_half(0, 0, 0, split_store=True)
    mm_half(0, 1, 0, split_store=True)
    t_duo("m1", 0, 0, (2, 3))
    t_pair("m2", 0, 3)
    t_pair("m2", 0, 2)
    mm_half(0, 0, 1, split_store=True)
    mm_half(0, 1, 1, split_store=True)
    mm_block(0, 2)
    mm_block(0, 3)
    t_pair("m1", 0, 1)

    # b1 m2 quads (8 singles) after b0 blocks 6..13; m1b1's first pairs
    # ride blocks 14/15 so no transpose bubble sits right before b1's
    # first matmul block
    b1_m2 = [("m2", 1, q, dc) for q in range(NQ) for dc in range(DC)]
    after_b0 = {
        5: [("m1", 0, 2)],
        8: [("m1", 0, 3)],
        14: [("m1", 1, 0)],
        15: [("m1", 1, 1)],
    }
    after_b1_blocks = {
        1: [("m1", 1, 2)],
        3: [("m1", 1, 3)],
    }

    for it in range(4, NT):
        mm_block(0, it)
        if 6 <= it <= 13:
            name, b, q, dc = b1_m2[it - 6]
            t_quad(name, b, q, dc)
        for name, b, q in after_b0.get(it, []):
            t_pair(name, b, q)

    for it in range(NT):
        # last row-block stores in halves on both rings to shorten the
        # final drain
        mm_block(1, it, split_store=(it == NT - 1))
        for name, b, q in after_b1_blocks.get(it, []):
            t_pair(name, b, q)


_NC_CACHE = None


def _build():
    global _NC_CACHE
    if _NC_CACHE is not None:
        return _NC_CACHE
    nc = bacc.Bacc(
        "TRN2", target_bir_lowering=False, debug=False, num_devices=NCORES
    )
    m1 = nc.dram_tensor("m1", [BPC, R1, D], F32R, kind="ExternalInput").ap()
    m2 = nc.dram_tensor("m2", [BPC, R2, D], F32R, kind="ExternalInput").ap()
    out = nc.dram_tensor("out", [BPC, R1, R2], F32, kind="ExternalOutput").ap()
    with tile.TileContext(nc) as tc:
        with ExitStack() as ctx:
            _build_tile_kernel(ctx, tc, m1, m2, out)
    nc.compile()
    _NC_CACHE = nc
    return nc


def kernel(matrix_1: np.ndarray, matrix_2: np.ndarray, **run_kwargs) -> np.ndarray:
    m1 = np.ascontiguousarray(np.asarray(matrix_1, dtype=np.float32))
    m2 = np.ascontiguousarray(np.asarray(matrix_2, dtype=np.float32))
    assert m1.shape == (B, R1, D) and m2.shape == (B, R2, D)

    nc = _build()
    in_maps = [
        {
            "m1": m1[i * BPC : (i + 1) * BPC],
            "m2": m2[i * BPC : (i + 1) * BPC],
        }
        for i in range(NCORES)
    ]
    res = run_bass_kernel_spmd(
        nc, in_maps, core_ids=list(range(NCORES)), **run_kwargs
    )
    out = np.empty((B, R1, R2), dtype=np.float32)
    for i in range(NCORES):
        out[i * BPC : (i + 1) * BPC] = res.results[i]["out"]
    if run_kwargs:
        kernel.last_result = res
    return out


# revision 38
# speedup vs baseline: 1.0590x; 1.0070x over previous
"""Batched matrix-attention scores kernel for Trainium2 (8 NeuronCores).

Computes scores[b, i, j] = sum_d m1[b, i, d] * m2[b, j, d]
  (i.e. jnp.einsum('bid,bjd->bij', matrix_1, matrix_2))
with B=16, R1=R2=2048, D=256, fp32 in/out.

Sharding: data-parallel over batch - 2 batches per core on 8 cores.

Per-core HBM traffic is 8 MiB of loads + 32 MiB of stores; one HWDGE
queue (spread over all 16 SDMA engines) sustains ~425 GB/s, so the
roofline is ~100 us + ~7 us engine-init preamble. The schedule keeps
the DMA queues continuously fed:

  - The sync queue gets only the three load chunks that gate the first
    stores, so it drains right as the first store's data stages; the
    rest of b0 rides the scalar queue. All b1 loads ride the gpsimd
    (SWDGE) queue, gated per-chunk by shared nat buffers (WAR on the
    partner b0 chunk's transposes) so they start ~14us and finish
    before heavy DVE evac traffic starves SWDGE descriptor generation.
  - m1 is loaded contiguously (4 consecutive DRAM rows per partition,
    4 KiB descriptor lines vs 1 KiB); the row permutation is absorbed
    by the store AP (partition stride 4 rows, 8 KiB lines kept). m2
    feeds the matmul j-columns so it stays in natural row order.
  - Operands are PE-transposed (matmul transpose mode with an on-chip
    identity, built right after the startup barrier) into
    D-on-partitions layout mT[d, dc, c]; transposes pack 4-to-a-PSUM-
    bank so one [128,512] copy evacuates a whole quad, and they chase
    the load chunks. Warmup transposes hold the PE HAM activity window
    busy so the ramp runs at K=8/8.
  - Each 128-row output tile takes 8 matmuls (2 d-chunks x 4 j-chunks
    of N=512, two 2-bank PSUM tiles); PSUM is evacuated split across
    DVE and ACT (one [128,1024] copy each) so store production stays
    above the DMA drain rate; output stored in 1 MiB blocks on the
    sync ring. Blocks 0-1 run and store in j-halves (gated on only
    half of m2T and two m1 transposes) and the last block stores in
    halves on both rings to shorten ramp and drain.
  - b1's transpose quads are trickled one-per-block between b0's later
    matmul blocks so the PE never lets the store queue drain.

Operands use dt.float32r (fp32 bits, full-rate single-pass PE matmul;
~2^-11 input mantissa truncation). Accumulation stays fp32 in PSUM.
"""

from contextlib import ExitStack

import numpy as np

import concourse.bass as bass
import concourse.mybir as mybir
import concourse.tile as tile
from concourse import bacc
from concourse.bass_utils import run_bass_kernel_spmd

F32 = mybir.dt.float32
F32R = mybir.dt.float32r

NCORES = 8
B, R1, R2, D = 16, 2048, 2048, 256
BPC = B // NCORES  # batches per core
P = 128
NJ_TILE = 512  # matmul free dim (one fp32 PSUM bank)
NJ = R2 // NJ_TILE  # j-chunks per row-block
NT = R1 // P  # 128-row tiles per batch
DC = D // P  # contraction chunks
NQ = NT // 4  # transpose quads (4 row-blocks) per (matrix, dc)
WARMUP_T = 10  # HAM warmup transposes


def _build_tile_kernel(ctx: ExitStack, tc: tile.TileContext, m1, m2, out):
    nc = tc.nc

    const_pool = ctx.enter_context(tc.tile_pool(name="const", bufs=1))
    # identity built on-chip right after the startup barrier (~3.5us) —
    # no DMA dependency, so HAM warmup transposes can start immediately
    ident = const_pool.tile([P, P], F32R)
    ident_f = const_pool.tile([P, P], F32, tag="identf", name="ident_f")
    nc.gpsimd.memset(ident_f, 1.0)
    nc.gpsimd.affine_select(
        out=ident_f,
        in_=ident_f,
        compare_op=mybir.AluOpType.is_equal,
        fill=0.0,
        base=0,
        # ident_f[x, y] = (x - y == 0) ? 1.0 : 0.0
        pattern=[[-1, P]],
        channel_multiplier=1,
    )
    nc.vector.tensor_copy(ident, ident_f)  # f32 -> f32r rounding copy

    nat_pool = ctx.enter_context(tc.tile_pool(name="nat", bufs=1))
    mt_pool = ctx.enter_context(tc.tile_pool(name="mt", bufs=1))
    tpsum = ctx.enter_context(tc.tile_pool(name="tpsum", bufs=2, space="PSUM"))
    mpsum = ctx.enter_context(tc.tile_pool(name="mpsum", bufs=3, space="PSUM"))
    outp = ctx.enter_context(tc.tile_pool(name="outp", bufs=8))

    # nat buffers are SHARED across batches in CROSS tags (A: m2b0+m1b1,
    # B: m1b0+m2b1, bufs=1): a b1 chunk-q load then carries a WAR
    # dependency on b0's quad-q transposes of the tag partner — a
    # scheduler-proof gate that releases b1 load traffic at ~13us (once
    # the ramp-critical b0 data has been consumed) but well before the
    # evac-heavy matmul phase where the SWDGE queue gets starved.
    #
    # m1 is loaded CONTIGUOUSLY (partition p holds 4 consecutive DRAM
    # rows 4p+t of its 512-row chunk -> 4 KiB descriptor lines instead
    # of 1 KiB); the resulting row permutation is absorbed by the store
    # AP (partition stride 4 rows), which keeps 8 KiB store lines. m2
    # feeds the matmul j-columns, so it must stay in natural row order
    # (1 KiB lines).
    # per-chunk nat tiles, all shaped [P, 4, D]; chunk q of (m2,b0) shares
    # a buffer with chunk q of (m1,b1) and vice versa, so each b1 chunk
    # load WAR-waits on exactly its partner chunk's 8 transposes
    nat = {}
    mt = {}
    for b in range(BPC):
        for name in ("m2", "m1"):
            grp = "a" if (name == "m2") == (b == 0) else "b"
            for q in range(NQ):
                nat[(name, b, q)] = nat_pool.tile(
                    [P, 4, D], F32R, tag=f"nat_{grp}_{q}", name=f"nat_{name}_{b}_{q}"
                )
            mt[(name, b)] = mt_pool.tile(
                [P, DC, R1], F32R, tag=f"mt_{name}_{b}", name=f"mt_{name}_{b}"
            )

    def load_chunk(eng, name, b, q, half=None):
        """One 512 KiB quad-chunk (4 row-blocks) of a matrix into nat.
        half=0/1 loads only the first/second pair of m1 t-slots (256 KiB)."""
        if name == "m2":
            eng.dma_start(
                nat[(name, b, q)],
                m2[b].rearrange("(o p) d -> p o d", p=P)[:, q * 4 : (q + 1) * 4, :],
            )
        elif half is None:
            eng.dma_start(
                nat[(name, b, q)],
                m1[b].rearrange("(q p t) d -> p q t d", p=P, t=4)[:, q],
            )
        else:
            sl = slice(half * 2, half * 2 + 2)
            eng.dma_start(
                nat[(name, b, q)][:, sl],
                m1[b].rearrange("(q p t) d -> p q t d", p=P, t=4)[:, q, sl],
            )

    t_toggle = [0]

    def t_quad(name, b, q, dc):
        """Transpose 4 row-blocks (one d-chunk) into one PSUM bank, then
        evacuate with a single [128,512] copy on alternating engines."""
        tp = tpsum.tile([P, NJ_TILE], F32R, tag="tp", name=f"tp_{name}_{b}_{q}_{dc}")
        for k in range(4):
            src = nat[(name, b, q)][:, k, dc * P : (dc + 1) * P]
            nc.tensor.transpose(tp[:, k * P : (k + 1) * P], src, ident)
        dst = mt[(name, b)][:, dc, q * NJ_TILE : (q + 1) * NJ_TILE]
        if t_toggle[0] % 2 == 0:
            nc.vector.tensor_copy(dst, tp)
        else:
            nc.scalar.copy(dst, tp)
        t_toggle[0] += 1

    def t_pair(name, b, q):
        for dc in range(DC):
            t_quad(name, b, q, dc)

    def t_duo(name, b, q, slots):
        """Transpose 2 row-slots for BOTH d-chunks into one PSUM bank
        (two [128,256] evac copies) — lets blk0/blk1 start after only
        half of m1's first chunk has landed."""
        tp = tpsum.tile([P, NJ_TILE], F32R, tag="tp", name=f"tpd_{name}_{b}_{q}_{slots[0]}")
        for di in range(DC):
            for si, k in enumerate(slots):
                src = nat[(name, b, q)][:, k, di * P : (di + 1) * P]
                nc.tensor.transpose(
                    tp[:, di * 2 * P + si * P : di * 2 * P + (si + 1) * P], src, ident
                )
        for di in range(DC):
            dst = mt[(name, b)][
                :, di, q * NJ_TILE + slots[0] * P : q * NJ_TILE + (slots[0] + 2) * P
            ]
            if t_toggle[0] % 2 == 0:
                nc.vector.tensor_copy(dst, tp[:, di * 2 * P : (di + 1) * 2 * P])
            else:
                nc.scalar.copy(dst, tp[:, di * 2 * P : (di + 1) * 2 * P])
            t_toggle[0] += 1

    stages = {}

    def mm_half(b, it, half, split_store=False):
        """Half of a 128-row output tile (j-halves of 1024): 4 matmuls into
        one 2-bank PSUM tile, evacuated on DVE (half 0) / ACT (half 1).
        split_store: store this half on its own (512 KiB); else the full
        1 MiB row-block is stored on sync once both halves are staged."""
        m2T, m1T = mt[("m2", b)], mt[("m1", b)]
        if (b, it) not in stages:
            stages[(b, it)] = outp.tile(
                [P, R2], F32, tag="stage", name=f"stage_{b}_{it}"
            )
        stage = stages[(b, it)]
        ps = mpsum.tile([P, 2 * NJ_TILE], F32, tag="mm", name=f"mps_{b}_{it}_{half}")
        for jl in range(2):
            jc = half * 2 + jl
            for dc in range(DC):
                nc.tensor.matmul(
                    ps[:, jl * NJ_TILE : (jl + 1) * NJ_TILE],
                    m1T[:, dc, it * P : (it + 1) * P],
                    m2T[:, dc, jc * NJ_TILE : (jc + 1) * NJ_TILE],
                    start=(dc == 0),
                    stop=(dc == DC - 1),
                )
        lo, hi = half * 2 * NJ_TILE, (half + 1) * 2 * NJ_TILE
        dst = stage[:, lo:hi]
        if half == 0:
            nc.vector.tensor_copy(dst, ps)
        else:
            nc.scalar.copy(dst, ps)
        # m1's contiguous load layout maps psum partition p to DRAM row
        # qq*512 + 4p + t -- the store AP un-permutes (8 KiB lines kept)
        qq, t = divmod(it, 4)
        orows = out[b].rearrange("(q p t) j -> p q t j", p=P, t=4)[:, qq, t, :]
        if split_store:
            eng = nc.scalar if (half == 1 and it == NT - 1 and b == BPC - 1) else nc.sync
            eng.dma_start(orows[:, lo:hi], stage[:, lo:hi])
        elif half == 1:
            nc.sync.dma_start(orows, stage)

    def mm_block(b, it, split_store=False):
        mm_half(b, it, 0, split_store)
        mm_half(b, it, 1, split_store)

    # ---- loads ----
    # The sync queue gets ONLY the three chunks gating the first stores
    # (m2 q0, m1 q0 first half, m2 q2) so it drains by the time the
    # first store's data is staged; the rest of b0 rides scalar, with
    # one chunk (m1 q3, not needed until block 12) on gpsimd. All b1
    # loads follow on the gpsimd (SWDGE) queue; the shared nat buffers
    # gate them (WAR) behind b0's transposes, so they start ~14us and
    # finish before the evac-heavy phase starves SWDGE.
    load_chunk(nc.sync, "m2", 0, 0)
    load_chunk(nc.sync, "m1", 0, 0, half=0)
    load_chunk(nc.sync, "m2", 0, 2)
    load_chunk(nc.sync, "m1", 0, 3)
    load_chunk(nc.scalar, "m2", 0, 1)
    load_chunk(nc.scalar, "m1", 0, 0, half=1)
    load_chunk(nc.scalar, "m2", 0, 3)
    load_chunk(nc.scalar, "m1", 0, 1)
    load_chunk(nc.scalar, "m1", 0, 2)
    # interleave b1's chunks by earliest need: m2b1 feeds transpose quads
    # from b0 block 7 on; m1b1 isn't needed until after b0 block 15
    for name, q in (
        ("m2", 0),
        ("m2", 1),
        ("m1", 0),
        ("m2", 2),
        ("m1", 1),
        ("m2", 3),
        ("m1", 2),
        ("m1", 3),
    ):
        load_chunk(nc.gpsimd, name, 1, q)

    # ---- HAM warmup: dummy transposes on the identity. Fillers are also
    # interleaved into the ramp (see below) so the PE activity window
    # stays busy across load-wait gaps and the first matmuls run at
    # K=8/8 instead of re-throttled 1.2 GHz. ----
    warm_n = [0]

    def fillers(n):
        for _ in range(n):
            wtp = tpsum.tile([P, NJ_TILE], F32R, tag="tp", name=f"warm_{warm_n[0]}")
            nc.tensor.transpose(wtp[:, 0:P], ident, ident)
            warm_n[0] += 1

    fillers(WARMUP_T)

    # ---- ramp: transposes chase the load chunks; blocks 0-1 run and
    # store in j-halves so the first store needs only half of m2T and
    # only m1's first two t-slots (one 256 KiB half-chunk). ----
    t_pair("m2", 0, 0)
    t_pair("m2", 0, 1)
    t_duo("m1", 0, 0, (0, 1))
    mm_half(0, 0, 0, split_store=True)
    mm_half(0, 1, 0, split_store=True)
    t_duo("m1", 0, 0, (2, 3))
    t_pair("m2", 0, 3)
    t_pair("m2", 0, 2)
    mm_half(0, 0, 1, split_store=True)
    mm_half(0, 1, 1, split_store=True)
    mm_block(0, 2)
    mm_block(0, 3)
    t_pair("m1", 0, 1)

    # b1 m2 quads (8 singles) after b0 blocks 6..13; m1b1's first pairs
    # ride blocks 14/15 so no transpose bubble sits right before b1's
    # first matmul block
    b1_m2 = [("m2", 1, q, dc) for q in range(NQ) for dc in range(DC)]
    after_b0 = {
        5: [("m1", 0, 2)],
        8: [("m1", 0, 3)],
        14: [("m1", 1, 0)],
        15: [("m1", 1, 1)],
    }
    after_b1_blocks = {
        1: [("m1", 1, 2)],
        3: [("m1", 1, 3)],
    }

    for it in range(4, NT):
        mm_block(0, it)
        if 6 <= it <= 13:
            name, b, q, dc = b1_m2[it - 6]
            t_quad(name, b, q, dc)
        for name, b, q in after_b0.get(it, []):
            t_pair(name, b, q)

    for it in range(NT):
        # last row-block stores in halves on both rings to shorten the
        # final drain
        mm_block(1, it, split_store=(it == NT - 1))
        for name, b, q in after_b1_blocks.get(it, []):
            t_pair(name, b, q)


_NC_CACHE = None


def _build():
    global _NC_CACHE
    if _NC_CACHE is not None:
        return _NC_CACHE
    nc = bacc.Bacc(
        "TRN2", target_bir_lowering=False, debug=False, num_devices=NCORES
    )
    m1 = nc.dram_tensor("m1", [BPC, R1, D], F32R, kind="ExternalInput").ap()
    m2 = nc.dram_tensor("m2", [BPC, R2, D], F32R, kind="ExternalInput").ap()
    out = nc.dram_tensor("out", [BPC, R1, R2], F32, kind="ExternalOutput").ap()
    with tile.TileContext(nc) as tc:
        with ExitStack() as ctx:
            _build_tile_kernel(ctx, tc, m1, m2, out)
    nc.compile()
    _NC_CACHE = nc
    return nc


def kernel(matrix_1: np.ndarray, matrix_2: np.ndarray, **run_kwargs) -> np.ndarray:
    m1 = np.ascontiguousarray(np.asarray(matrix_1, dtype=np.float32))
    m2 = np.ascontiguousarray(np.asarray(matrix_2, dtype=np.float32))
    assert m1.shape == (B, R1, D) and m2.shape == (B, R2, D)

    nc = _build()
    in_maps = [
        {
            "m1": m1[i * BPC : (i + 1) * BPC],
            "m2": m2[i * BPC : (i + 1) * BPC],
        }
        for i in range(NCORES)
    ]
    res = run_bass_kernel_spmd(
        nc, in_maps, core_ids=list(range(NCORES)), **run_kwargs
    )
    out = np.empty((B, R1, R2), dtype=np.float32)
    for i in range(NCORES):
        out[i * BPC : (i + 1) * BPC] = res.results[i]["out"]
    if run_kwargs:
        kernel.last_result = res
    return out
